# revision 1
# baseline (speedup 1.0000x reference)
"""Causal self-attention (B=2, T=2048, D=1024, NH=16) on 8 Trainium2 NeuronCores.

v2: single interleaved instruction stream. The ACT engine runs exp
exclusively; all QKV-projection / V-compute / output-projection matmul
"jobs" are decomposed into ~1-3.5us chunks and sprinkled between
attention score groups so the PE fills the gaps that exp latency leaves.

Sharding: core c handles batch b = c // 4 and heads [4*(c%4), 4*(c%4)+4)
(same in_maps as baseline). Numerics are operand- and order-identical to
the baseline kernel (f32r matmuls, same accumulation order), so the
relative error is unchanged.

PSUM (8 banks): psS 2x[128,1024] attention scores (4) | psO 2x[128,512]
per-(head, q-quarter) PV accumulators (2) | psX 2x[128,512] QK/V/proj
jobs (2).

v7: the per-(head-pair, quarter) PV accumulator is copied PSUM->SBUF with
one DVE tensor_copy before the normalize chain runs, releasing the PSUM
bank ~2.5us earlier for the next quarter's PV matmuls.
"""

import contextlib

import numpy as np

import concourse.bass as bass
import concourse.mybir as mybir
import concourse.tile as tile
from concourse import bacc
from concourse.bass_utils import run_bass_kernel_spmd

B, T, D = 2, 2048, 1024
NH, HD = 16, 64
NCORES = 8
HPC = 4                 # heads per core
HDIM = HPC * HD         # 256 qkv dims per core
KCH = D // 128          # 8 contraction chunks
F32 = mybir.dt.float32
F32R = mybir.dt.float32r
EXP = mybir.ActivationFunctionType.Exp
MULT = mybir.AluOpType.mult

_NC = None


def _build(debug=False, reps=1, staggered=False, no_out=False, hoist_in=False):
    nc = bacc.Bacc()
    xT = nc.declare_dram_parameter("xT", [D, T], F32R, isOutput=False)
    Wa = nc.declare_dram_parameter("Wa", [D, 3 * HDIM], F32R, isOutput=False)
    Wp = nc.declare_dram_parameter("Wp", [HDIM, D], F32R, isOutput=False)
    bqk = nc.declare_dram_parameter("bqk", [128, 4], F32, isOutput=False)
    bv = nc.declare_dram_parameter("bv", [1, HDIM], F32R, isOutput=False)
    tri = nc.declare_dram_parameter("tri", [128, 128], F32R, isOutput=False)
    out = nc.declare_dram_parameter("out", [D, T], F32, isOutput=True)

    with tile.TileContext(nc) as tc:
        with (
            tc.tile_pool(name="persist", bufs=1) as pp,
            tc.tile_pool(name="psS", bufs=2, space="PSUM") as psS,
            tc.tile_pool(name="psO", bufs=2, space="PSUM") as psO,
            tc.tile_pool(name="psX", bufs=2, space="PSUM") as psX,
        ):
            # [partition, chunk, token]: chunks 0-1 = Q^T (h01, h23), 2-3 = K^T
            qkT = pp.tile([128, 4, T], F32R)
            # V1 blocks per (t-chunk, head-PAIR), 194-col pitch:
            #   even head lhsT = cols 0:65   [V(64) | 1]          -> denom row 64
            #   odd  head lhsT = cols 66:194 [z32 | 1 | z31 | V]  -> denom row 32,
            #                                                        V rows 64:128
            # (matmul tile_position: out base partition must be 0 for >64 rows)
            vsb = pp.tile([128, 16, 2, 194], F32R)
            yT = pp.tile([128, 2, T], F32R)
            wp = pp.tile([128, 2, D], F32R)
            trisb = pp.tile([128, 128], F32R)
            bqksb = pp.tile([128, 4], F32)
            bvsb = pp.tile([1, HDIM], F32R)
            onesr = pp.tile([1, 128], F32R)

            nc.sync.dma_start(trisb[:], tri[:])
            nc.sync.dma_start(bqksb[:], bqk[:])
            nc.sync.dma_start(bvsb[:], bv[:])
            for c in range(2):
                nc.sync.dma_start(wp[:, c, :], Wp[c * 128 : (c + 1) * 128, :])
            # weights stay resident across iterations (like Wp): load Wa once
            ws = []
            for k in range(KCH):
                wt = pp.tile([128, 3 * HDIM], F32R, name=f"w{k}")
                eng = nc.sync if k % 2 == 0 else nc.scalar
                eng.dma_start(wt[:], Wa[k * 128 : (k + 1) * 128, :])
                ws.append(wt)
            # span-0 x columns live in a persistent prefetch buffer: loaded
            # here for iteration 0 and refreshed mid-body for the next
            # iteration, so the first QK/V jobs start right after the loop
            # barrier instead of waiting ~30us for x DMAs
            xp = []
            for k in range(KCH):
                xt = pp.tile([128, 512], F32R, name=f"xp{k}")
                eng = nc.sync if k % 2 == 0 else nc.scalar
                eng.dma_start(xt[:], xT[k * 128 : (k + 1) * 128, 0:512])
                xp.append(xt)
            nc.gpsimd.memset(onesr[:].bitcast(F32), 1.0)
            nc.gpsimd.memset(vsb[:].bitcast(F32), 0.0)
            nc.gpsimd.memset(vsb[:, :, :, 64:65].bitcast(F32), 1.0)   # even: col 64
            nc.gpsimd.memset(vsb[:, :, :, 98:99].bitcast(F32), 1.0)   # odd: col 66+32

            loop_ctx = (
                tc.For_i(0, reps, 1, staggered_reset=staggered)
                if reps > 1
                else contextlib.nullcontext()
            )
            with loop_ctx, contextlib.ExitStack() as st:
                inA = st.enter_context(tc.tile_pool(name="inA", bufs=1))
                wk = st.enter_context(tc.tile_pool(name="wk", bufs=3))
                nm = st.enter_context(tc.tile_pool(name="nm", bufs=2))
                obp = st.enter_context(tc.tile_pool(name="obp", bufs=1))

                # ---- input DMAs: span-0 columns come from the persistent
                # prefetch buffer; the in-loop transfers cover cols 512:2048
                # (8 x 0.75 MB, alternating the two HWDGE queues)
                xs = []
                for k in range(KCH):
                    xt = inA.tile([128, T - 512], F32R, tag=f"x{k}", name=f"x{k}")
                    eng = nc.sync if k % 2 == 0 else nc.scalar
                    eng.dma_start(xt[:], xT[k * 128 : (k + 1) * 128, 512:T])
                    xs.append(xt)

                def xcol(k, c0, c1):
                    # x columns [c0, c1) (never crosses the 512 boundary)
                    if c1 <= 512:
                        return xp[k][:, c0:c1]
                    return xs[k][:, c0 - 512 : c1 - 512]

                def x_refresh():
                    # reload span-0 columns for the NEXT iteration (emitted
                    # after this iteration's last xp reader)
                    for k in range(KCH):
                        eng = nc.sync if k % 2 == 0 else nc.scalar
                        eng.dma_start(xp[k][:], xT[k * 128 : (k + 1) * 128, 0:512])

                # ---- job emitters (each grabs one psX slot) ----
                def qk_job(sp, m):
                    # Q/K chunk m over token span sp: 8 accumulating matmuls
                    acc = psX.tile([128, 512], F32, tag="x", name="qkacc")
                    for k in range(KCH):
                        nc.tensor.matmul(
                            acc[:],
                            ws[k][:, m * 128 : (m + 1) * 128],
                            xcol(k, sp * 512, (sp + 1) * 512),
                            start=(k == 0),
                            stop=(k == KCH - 1),
                        )
                    nc.vector.tensor_scalar_add(
                        qkT[:, m, sp * 512 : (sp + 1) * 512], acc[:], bqksb[:, m : m + 1]
                    )

                def v_job(t):
                    acc = psX.tile([128, 512], F32, tag="x", name="vacc")[:, 0:HDIM]
                    ks = [(t + i) % KCH for i in range(KCH)]
                    for i, k in enumerate(ks):
                        nc.tensor.matmul(
                            acc[:],
                            xcol(k, t * 128, (t + 1) * 128),
                            ws[k][:, 2 * HDIM : 3 * HDIM],
                            start=(i == 0),
                            stop=False,
                        )
                    nc.tensor.matmul(
                        acc[:], onesr[0:1, :], bvsb[0:1, :], start=False, stop=True
                    )
                    # scatter the 4 heads into their V1 blocks (2 strided copies)
                    src_e = acc[:, 0:192].rearrange("p (h d) -> p h d", d=64)[:, ::2]
                    nc.vector.tensor_copy(vsb[:, t, :, 0:64], src_e)
                    src_o = acc[:, 64:256].rearrange("p (h d) -> p h d", d=64)[:, ::2]
                    nc.vector.tensor_copy(vsb[:, t, :, 130:194], src_o)

                obts = {}

                def d_job(sp, m):
                    # output projection block [m, sp]: contract both 128-chunks
                    if m % 4 == 0:
                        obts[(sp, m // 4)] = obp.tile(
                            [128, 4, 512], F32, tag="ob", name="obt"
                        )
                    acc = psX.tile([128, 512], F32, tag="x", name="dacc")
                    for c in range(2):
                        nc.tensor.matmul(
                            acc[:],
                            wp[:, c, m * 128 : (m + 1) * 128],
                            yT[:, c, sp * 512 : (sp + 1) * 512],
                            start=(c == 0),
                            stop=(c == 1),
                        )
                    nc.vector.tensor_copy(obts[(sp, m // 4)][:, m % 4, :], acc[:])

                def d_flush(sp, half):
                    # one 1 MB DMA per 4-row-block half-span: dram side
                    # iterates [p, m, c]
                    if no_out:
                        return
                    dst = out[
                        half * 512 : (half + 1) * 512, sp * 512 : (sp + 1) * 512
                    ].rearrange("(m p) c -> p m c", p=128)
                    eng = nc.sync if (sp + half) % 2 == 0 else nc.scalar
                    eng.dma_start(dst, obts[(sp, half)][:])

                # ---- attention segment: head pair, one 512-token q-quarter.
                # extras: job thunks popped one per score-group (leftovers run
                # at segment end).
                def attn_seg(pair, qb, extras):
                    nki = 4 * qb + 4
                    ocs = {}
                    ohs = {}
                    for j in pair:
                        ohs[j] = psO.tile([128, 512], F32, tag="o", name=f"oh{j}")
                    groups = [
                        tuple(ki for ki in (g, g + 1) if ki < nki)
                        for g in range(0, nki, 2)
                    ]
                    for grp in groups:
                        if extras:
                            extras.pop(0)()
                        # pack the 1-2 pieces contiguously (second piece goes
                        # at offset w0 when it fits in the same PSUM bank):
                        # exp covers [0, ew) with no stale gap and no waste
                        offs, ws = [], []
                        for ki in grp:
                            s0 = max(128 * ki, 512 * qb)
                            w = 512 * (qb + 1) - s0
                            if not offs:
                                offs.append(0)
                            else:
                                offs.append(ws[0] if ws[0] + w <= 512 else 512)
                            ws.append(w)
                        ew = offs[-1] + ws[-1]
                        sps, psbs = {}, {}
                        for j in pair:
                            po = 64 * (j % 2)
                            qc, kc = j // 2, 2 + j // 2
                            sp_t = psS.tile([128, 1024], F32, tag="s", name="sps")
                            sps[j] = sp_t
                            for gi, ki in enumerate(grp):
                                s0 = max(128 * ki, 512 * qb)
                                nc.tensor.matmul(
                                    sp_t[:, offs[gi] : offs[gi] + ws[gi]],
                                    qkT[po : po + 64, kc, 128 * ki : 128 * ki + 128],
                                    qkT[po : po + 64, qc, s0 : s0 + ws[gi]],
                                    start=True,
                                    stop=True,
                                )
                        for j in pair:
                            psb = wk.tile([128, 1024], F32R, tag="p", name="psb")
                            psbs[j] = psb
                            nc.scalar.activation(
                                psb[:, :ew], sps[j][:, :ew], EXP, scale=0.125
                            )
                            for gi, ki in enumerate(grp):
                                if 128 * ki >= 512 * qb:  # diagonal block: mask
                                    nc.gpsimd.tensor_tensor(
                                        psb[:, offs[gi] : offs[gi] + 128],
                                        psb[:, offs[gi] : offs[gi] + 128],
                                        trisb[:],
                                        MULT,
                                    )
                        for j in pair:
                            ph = j // 2
                            if j % 2 == 0:
                                vcols, orows = (0, 65), (0, 65)
                            else:
                                vcols, orows = (66, 194), (0, 128)
                            for gi, ki in enumerate(grp):
                                s0 = max(128 * ki, 512 * qb)
                                nc.tensor.matmul(
                                    ohs[j][
                                        orows[0] : orows[1],
                                        s0 - 512 * qb : s0 - 512 * qb + ws[gi],
                                    ],
                                    vsb[:, ki, ph, vcols[0] : vcols[1]],
                                    psbs[j][:, offs[gi] : offs[gi] + ws[gi]],
                                    start=(ki == 0),
                                    stop=(ki == nki - 1),
                                    skip_group_check=True,
                                )
                    for fn in extras:
                        fn()
                    extras.clear()
                    # normalize: y^T = O^T * (1/denom) with denom from the V1
                    # ones column (partition 64 even head / 32 odd head)
                    for j in pair:
                        po = 64 * (j % 2)
                        dp = 64 if j % 2 == 0 else 32
                        # one copy frees the PSUM oh slot for the next
                        # quarter ~2.5us earlier than the normalize chain
                        oc = nm.tile([128, 512], F32, tag="oc", name="oc")
                        # full-partition copy: a DVE pattern starting at
                        # partition 32 may span at most 32 partitions
                        nc.vector.tensor_copy(oc[:, :], ohs[j][:, :])
                        ocs[j] = oc
                    for j in pair:
                        po = 64 * (j % 2)
                        qc = j // 2
                        dp = 64 if j % 2 == 0 else 32
                        drec = nm.tile([1, 512], F32, tag="dc", name="drec")
                        nc.vector.reciprocal(drec[0:1, :], ocs[j][dp : dp + 1, :])
                        rb = nm.tile([128, 512], F32R, tag="rb", name="rb")
                        nc.gpsimd.partition_broadcast(
                            rb[:], drec[0:1, :].bitcast(F32R)
                        )
                        nc.vector.tensor_tensor(
                            yT[po : po + 64, qc, qb * 512 : (qb + 1) * 512],
                            ocs[j][po : po + 64, :],
                            rb[po : po + 64, :],
                            MULT,
                        )

                # ---- the schedule ----
                P0, P1 = (0, 1), (2, 3)
                qk_j = lambda sp, m: (lambda: qk_job(sp, m))
                v_j = lambda t: (lambda: v_job(t))
                d_j = lambda sp, m: (lambda: d_job(sp, m))

                # NOTE extras pop one per score-group, leftovers at segment
                # end — so a job must be listed at least one group before
                # the first matmul that consumes its output.
                qk_job(0, 0)
                qk_job(0, 2)
                v_job(0)
                v_job(1)
                v_job(2)
                v_job(3)
                attn_seg(P0, 0, [qk_j(1, 0), qk_j(1, 2), v_j(4), v_j(5)])
                attn_seg(P0, 1, [v_j(6), v_j(7), qk_j(2, 0), qk_j(2, 2)])
                attn_seg(P0, 2, [v_j(8), v_j(9), v_j(10), v_j(11),
                                 qk_j(3, 0), qk_j(3, 2)])
                attn_seg(P0, 3, [v_j(12), v_j(13), v_j(14), v_j(15),
                                 qk_j(0, 1), qk_j(0, 3), qk_j(1, 1), qk_j(1, 3)])
                def d_extras(sp):
                    ex = [d_j(sp, m) for m in range(8)]
                    ex.insert(4, lambda: d_flush(sp, 0))
                    ex.append(lambda: d_flush(sp, 1))
                    return ex

                x_refresh()  # all span-0 readers retired after P0-qb3
                attn_seg(P1, 0, [qk_j(2, 1), qk_j(2, 3), qk_j(3, 1), qk_j(3, 3)])
                attn_seg(P1, 1, d_extras(0))
                attn_seg(P1, 2, d_extras(1))
                attn_seg(P1, 3, d_extras(2))
                for m in range(4):
                    d_job(3, m)
                d_flush(3, 0)
                for m in range(4, 8):
                    d_job(3, m)
                d_flush(3, 1)

    nc.compile()
    return nc


def _get_nc():
    global _NC
    if _NC is None:
        _NC = _build()
    return _NC


def _make_in_maps(x, Wqkv, bqkv, Wproj):
    x = np.ascontiguousarray(np.asarray(x, np.float32))
    Wqkv = np.asarray(Wqkv, np.float32)
    bqkv = np.asarray(bqkv, np.float32)
    Wproj = np.asarray(Wproj, np.float32)
    tri = np.triu(np.ones((128, 128), np.float32))  # tri[k, q] = q >= k
    in_maps = []
    for c in range(NCORES):
        b = c // 4
        cs = (c % 4) * HDIM
        ce = cs + HDIM
        Wa = np.concatenate(
            [Wqkv[:, cs:ce], Wqkv[:, D + cs : D + ce], Wqkv[:, 2 * D + cs : 2 * D + ce]],
            axis=1,
        )
        bqk_c = np.concatenate([bqkv[cs:ce], bqkv[D + cs : D + ce]])
        in_maps.append(
            {
                "xT": np.ascontiguousarray(x[b].T),
                "Wa": np.ascontiguousarray(Wa),
                "Wp": np.ascontiguousarray(Wproj[cs:ce, :]),
                "bqk": np.ascontiguousarray(bqk_c.reshape(4, 128).T),
                "bv": np.ascontiguousarray(bqkv[2 * D + cs : 2 * D + ce].reshape(1, HDIM)),
                "tri": tri,
            }
        )
    return in_maps


def _run(in_maps, **kwargs):
    nc = _get_nc()
    return run_bass_kernel_spmd(nc, in_maps, core_ids=list(range(NCORES)), **kwargs)


def kernel(x, Wqkv, bqkv, Wproj, bproj):
    in_maps = _make_in_maps(x, Wqkv, bqkv, Wproj)
    res = _run(in_maps)
    bproj = np.asarray(bproj, np.float32)
    outp = np.zeros((B, T, D), np.float32)
    for c in range(NCORES):
        outp[c // 4] += res.results[c]["out"].T
    outp += bproj[None, None, :]
    return outp

